# Initial kernel scaffold
#
"""Trainium2 Bass kernel for nn_DGN2_70428873720402 (gnn_message_passing).

Math (per batch b):
  surp_t  = tanh(sigma * mean_d |(x - ema_mean)/std|)           (B,T)
  K_t     = clip(round(2 + 14*surp), 0, 16)
  sim     = cosine-similarity(x_t, x_s), strictly causal (s < t)
  A[t,s]  = 1 iff sim[t,s] among top-K_t of row t               (threshold form)
  msg     = (A @ x) / max(min(K_t, t), 1)
  out     = gelu((mix*x + (1-mix)*msg)*gain + bias) * scale

Sharding: 8 cores = 2 batches x 4 row-stripes.  Core (b, j) owns global rows
t = 512*i + j + 4*m  (slot i in 0..7, m in 0..127) - every core has the same
slot widths W_i = 512*(i+1), so one SPMD program serves all 8 cores; the
stripe offset j only enters through host-prepared data (pre-gathered row
tensors, the diagonal staircase mask, per-row t values).

Per-row top-K via the DVE Max8 ISA: max -> match_replace -> max gives the
sorted top-16; the K_t-th value (selected with an iota mask) is the
threshold; A = (sim >= theta) & causal.  theta is floored at -1e8 so rows
with t <= K_t select exactly their t causal predecessors (mask slots = -1e9).
Ranking key: sim'[t,s] = (x_t . x_s) / ||x_s|| - the 1/||x_t|| row factor
cannot change a within-row ranking, so only columns are normalized (fused as
a postscale on the PE output).
"""

import sys

sys.path.insert(0, "/opt/trn_rl_repo")

import numpy as np
from contextlib import ExitStack

import concourse.bass as bass
import concourse.bacc as bacc
import concourse.mybir as mybir
import concourse.tile as tile

f32 = mybir.dt.float32
bf16 = mybir.dt.bfloat16
Alu = mybir.AluOpType
Act = mybir.ActivationFunctionType

NEG_BIG = -1.0e9
TH_FLOOR = -1.0e8
K_HIGH, K_LOW = 16, 2


def build_program(T=4096, D=1024, groups=((0, 1, 2, 3), (4, 5, 6, 7)), gelu_func=None):
    if gelu_func is None:
        gelu_func = Act.Gelu
    NSLOT = T // 512
    KD = D // 128
    R = NSLOT * 128
    HC = 256
    NHC = T // HC
    DH = D // 2

    nc = bacc.Bacc("TRN2", target_bir_lowering=False, debug=False)

    xT = nc.dram_tensor("xT", [D, T], f32, kind="ExternalInput")
    xrT = nc.dram_tensor("xrT", [D, R], f32, kind="ExternalInput")
    xnat = nc.dram_tensor("xnat", [T, D], f32, kind="ExternalInput")
    xrn = nc.dram_tensor("xrn", [R, D], f32, kind="ExternalInput")
    invstd_b = nc.dram_tensor("invstd_b", [128, D], f32, kind="ExternalInput")
    negm_b = nc.dram_tensor("negm_b", [128, D], f32, kind="ExternalInput")
    g1_b = nc.dram_tensor("g1_b", [128, D], f32, kind="ExternalInput")
    g2_b = nc.dram_tensor("g2_b", [128, D], f32, kind="ExternalInput")
    bias_b = nc.dram_tensor("bias_b", [128, D], f32, kind="ExternalInput")
    dmask = nc.dram_tensor("dmask", [128, 512], f32, kind="ExternalInput")
    iota16 = nc.dram_tensor("iota16", [128, 16], f32, kind="ExternalInput")
    trow = nc.dram_tensor("trow", [128, NSLOT], f32, kind="ExternalInput")
    sc = nc.dram_tensor("sc", [128, 2], f32, kind="ExternalInput")
    out = nc.dram_tensor("out", [R, D], f32, kind="ExternalOutput")

    xT_v = xT.rearrange("(k p) t -> p k t", p=128)
    xrT_v = xrT.rearrange("(k p) m -> p k m", p=128)
    xnat_v = xnat.rearrange("(a p) d -> p a d", p=128)

    with tile.TileContext(nc) as tc, ExitStack() as ctx:
        prm = ctx.enter_context(tc.tile_pool(name="prm", bufs=1))
        ptiny = ctx.enter_context(tc.tile_pool(name="ptiny", bufs=1))

        # ---- global params ----
        t_g1 = prm.tile([128, D], f32, tag="g1")
        nc.sync.dma_start(t_g1[:], g1_b[:])
        t_g2 = prm.tile([128, D], f32, tag="g2")
        nc.sync.dma_start(t_g2[:], g2_b[:])
        t_bias = prm.tile([128, D], f32, tag="bias")
        nc.sync.dma_start(t_bias[:], bias_b[:])
        t_dmask = prm.tile([128, 512], f32, tag="dmask")
        nc.sync.dma_start(t_dmask[:], dmask[:])
        t_iota = prm.tile([128, 16], f32, tag="iota")
        nc.sync.dma_start(t_iota[:], iota16[:])
        t_trow = prm.tile([128, NSLOT], f32, tag="trow")
        nc.sync.dma_start(t_trow[:], trow[:])
        t_sc = prm.tile([128, 2], f32, tag="sc")
        nc.sync.dma_start(t_sc[:], sc[:])
        t_ones = prm.tile([128, 1], f32, tag="ones")
        nc.vector.memset(t_ones[:], 1.0)
        t_ones_row = prm.tile([1, 128], f32, tag="ones_row")
        nc.vector.memset(t_ones_row[:], 1.0)

        # ---- phase 1: per-slot stats -> kmask, rdeg ----
        kmasks, rdegs = {}, {}
        with tc.tile_pool(name="pstat", bufs=2) as pstat:
            t_invstd = pstat.tile([128, D], f32, tag="invstd", bufs=1)
            nc.sync.dma_start(t_invstd[:], invstd_b[:])
            t_negm = pstat.tile([128, D], f32, tag="negm", bufs=1)
            nc.sync.dma_start(t_negm[:], negm_b[:])
            for i in range(NSLOT):
                xs = pstat.tile([128, D], f32, tag="xs")
                nc.sync.dma_start(xs[:], xrn[128 * i : 128 * (i + 1), :])
                z = pstat.tile([128, D], f32, tag="z")
                nc.vector.tensor_tensor(out=z[:], in0=xs[:], in1=t_invstd[:], op=Alu.mult)
                nc.vector.tensor_tensor(out=z[:], in0=z[:], in1=t_negm[:], op=Alu.add)
                scr = pstat.tile([128, D], f32, tag="scr")
                sabs = ptiny.tile([128, 1], f32, tag=f"sabs{i}", name=f"sabs{i}")
                nc.scalar.activation(scr[:], z[:], Act.Abs, accum_out=sabs[:])
                surp = ptiny.tile([128, 1], f32, tag=f"surp{i}", name=f"surp{i}")
                nc.scalar.activation(surp[:], sabs[:], Act.Tanh, scale=t_sc[:, 0:1])
                kraw = ptiny.tile([128, 1], f32, tag=f"kraw{i}", name=f"kraw{i}")
                nc.vector.tensor_scalar(
                    out=kraw[:], in0=surp[:], scalar1=float(K_HIGH - K_LOW),
                    scalar2=float(K_LOW), op0=Alu.mult, op1=Alu.add,
                )
                km05 = ptiny.tile([128, 1], f32, tag=f"km05_{i}", name=f"km05_{i}")
                nc.vector.tensor_scalar(
                    out=km05[:], in0=kraw[:], scalar1=0.5, scalar2=None, op0=Alu.subtract
                )
                km15 = ptiny.tile([128, 1], f32, tag=f"km15_{i}", name=f"km15_{i}")
                nc.vector.tensor_scalar(
                    out=km15[:], in0=kraw[:], scalar1=1.5, scalar2=None, op0=Alu.subtract
                )
                m1 = ptiny.tile([128, 16], f32, tag=f"m1_{i}", name=f"m1_{i}")
                nc.vector.tensor_scalar(
                    out=m1[:], in0=t_iota[:], scalar1=km05[:], scalar2=None, op0=Alu.is_le
                )
                m2 = ptiny.tile([128, 16], f32, tag=f"m2_{i}", name=f"m2_{i}")
                nc.vector.tensor_scalar(
                    out=m2[:], in0=t_iota[:], scalar1=km15[:], scalar2=None, op0=Alu.is_gt
                )
                kmask = ptiny.tile([128, 16], f32, tag=f"kmask{i}", name=f"kmask{i}")
                nc.vector.tensor_tensor(out=kmask[:], in0=m1[:], in1=m2[:], op=Alu.mult)
                kmasks[i] = kmask
                kss = ptiny.tile([128, 16], f32, tag=f"kss{i}", name=f"kss{i}")
                km1 = ptiny.tile([128, 1], f32, tag=f"km1_{i}", name=f"km1_{i}")
                nc.vector.tensor_tensor(
                    out=kss[:], in0=kmask[:], in1=t_iota[:], op=Alu.mult
                )
                nc.vector.tensor_reduce(
                    out=km1[:], in_=kss[:], axis=mybir.AxisListType.X, op=Alu.add
                )
                deg = ptiny.tile([128, 1], f32, tag=f"deg{i}", name=f"deg{i}")
                nc.vector.tensor_scalar(
                    out=deg[:], in0=km1[:], scalar1=1.0, scalar2=t_trow[:, i : i + 1],
                    op0=Alu.add, op1=Alu.min,
                )
                nc.vector.tensor_scalar(
                    out=deg[:], in0=deg[:], scalar1=1.0, scalar2=None, op0=Alu.max
                )
                rdeg = ptiny.tile([128, 1], f32, tag=f"rdeg{i}", name=f"rdeg{i}")
                nc.vector.reciprocal(rdeg[:], deg[:])
                rdegs[i] = rdeg

        rnrows = {}  # cached per-half-chunk 1/||x_s|| rows [1, HC]
        for gidx, slots in enumerate(groups):
            gmax = max(slots)
            n_hc = 2 * (gmax + 1)
            with ExitStack() as gctx:
                pat = gctx.enter_context(
                    tc.tile_pool(name=f"pat{gidx}", bufs=1)
                )
                ats = {}
                thetas = {}
                # ---------- sim + topk + A ----------
                with ExitStack() as sctx:
                    psim = sctx.enter_context(tc.tile_pool(name=f"psim{gidx}", bufs=1))
                    pstream = sctx.enter_context(
                        tc.tile_pool(name=f"pstream{gidx}", bufs=3)
                    )
                    prep = sctx.enter_context(tc.tile_pool(name=f"prep{gidx}", bufs=1))
                    pa = sctx.enter_context(tc.tile_pool(name=f"pa{gidx}", bufs=1))
                    pps = sctx.enter_context(
                        tc.tile_pool(name=f"pps{gidx}", bufs=2, space="PSUM")
                    )
                    ppsn = sctx.enter_context(
                        tc.tile_pool(name=f"ppsn{gidx}", bufs=2, space="PSUM")
                    )
                    lhs = {}
                    for k, i in enumerate(slots):
                        lt = psim.tile(
                            [128, KD, 128], f32, tag=f"lhs{k}", name=f"lhs{i}"
                        )
                        nc.sync.dma_start(lt[:], xrT_v[:, :, 128 * i : 128 * (i + 1)])
                        lhs[i] = lt
                    simbufs = {}
                    for k, i in enumerate(slots):
                        simbufs[i] = psim.tile(
                            [128, 512 * (i + 1)], f32, tag=f"sim{k}", name=f"sim{i}"
                        )
                    for hc in range(n_hc):
                        c0 = HC * hc
                        rhsc = pstream.tile([128, KD, HC], f32, tag="stream")
                        nc.sync.dma_start(rhsc[:], xT_v[:, :, c0 : c0 + HC])
                        if hc not in rnrows:
                            sq = pstream.tile(
                                [128, KD, HC], f32, tag="stream", name="sq"
                            )
                            nc.scalar.activation(sq[:], rhsc[:], Act.Square)
                            psn = ppsn.tile([1, HC], f32, tag="psn")
                            for dk in range(KD):
                                nc.tensor.matmul(
                                    psn[:], t_ones[:], sq[:, dk, :],
                                    start=(dk == 0), stop=(dk == KD - 1),
                                )
                            rln = ptiny.tile([1, HC], f32, tag="rln", name="rln")
                            nc.scalar.activation(rln[:], psn[:], Act.Ln)
                            rnr = prm.tile([1, HC], f32, tag=f"rnr{hc}", name=f"rnr{hc}")
                            nc.scalar.activation(rnr[:], rln[:], Act.Exp, scale=-0.5)
                            # one Newton step: r = r0*(1.5 - 0.5*v*r0^2)
                            nw = ptiny.tile([1, HC], f32, tag="nw", name="nw")
                            nc.vector.tensor_tensor(
                                out=nw[:], in0=rnr[:], in1=rnr[:], op=Alu.mult
                            )
                            nc.vector.tensor_tensor(
                                out=nw[:], in0=nw[:], in1=psn[:], op=Alu.mult
                            )
                            nc.vector.tensor_scalar(
                                out=nw[:], in0=nw[:], scalar1=-0.5, scalar2=1.5,
                                op0=Alu.mult, op1=Alu.add,
                            )
                            nc.vector.tensor_tensor(
                                out=rnr[:], in0=rnr[:], in1=nw[:], op=Alu.mult
                            )
                            rnrows[hc] = rnr
                        psb = ppsn.tile([128, HC], f32, tag="psb")
                        nc.tensor.matmul(
                            psb[:], t_ones_row[:], rnrows[hc][:], start=True, stop=True
                        )
                        rnb = pstream.tile([128, HC], f32, tag="rnb", name="rnb")
                        nc.scalar.activation(rnb[:], psb[:], Act.Copy)
                        for i in slots:
                            if 512 * (i + 1) <= c0:
                                continue
                            psg = pps.tile([128, HC], f32, tag="psg")
                            for dk in range(KD):
                                nc.tensor.matmul(
                                    psg[:], lhs[i][:, dk, :], rhsc[:, dk, :],
                                    start=(dk == 0), stop=(dk == KD - 1),
                                )
                            seg = simbufs[i][:, c0 : c0 + HC]
                            nc.vector.tensor_tensor(
                                out=seg, in0=psg[:], in1=rnb[:], op=Alu.mult
                            )
                            if hc // 2 == i:
                                dseg = t_dmask[:, (hc % 2) * HC : (hc % 2) * HC + HC]
                                nc.vector.tensor_tensor(
                                    out=seg, in0=seg, in1=dseg, op=Alu.add
                                )
                            if hc == 2 * i + 1:
                                W = 512 * (i + 1)
                                sb = simbufs[i]
                                top16 = ptiny.tile(
                                    [128, 16], f32, tag=f"top16_{i}", name=f"top16_{i}"
                                )
                                nc.vector.max(top16[:, 0:8], sb[:])
                                rep = prep.tile(
                                    [128, 512 * NSLOT], f32, tag="rep", name="rep"
                                )
                                nc.vector.match_replace(
                                    rep[:, :W], top16[:, 0:8], sb[:], NEG_BIG
                                )
                                nc.vector.max(top16[:, 8:16], rep[:, :W])
                                tts = ptiny.tile(
                                    [128, 16], f32, tag=f"tts{i}", name=f"tts{i}"
                                )
                                th = ptiny.tile(
                                    [128, 1], f32, tag=f"th{i}", name=f"th{i}"
                                )
                                nc.vector.tensor_tensor(
                                    out=tts[:], in0=top16[:], in1=kmasks[i][:],
                                    op=Alu.mult,
                                )
                                nc.vector.tensor_reduce(
                                    out=th[:], in_=tts[:], axis=mybir.AxisListType.X,
                                    op=Alu.add,
                                )
                                nc.vector.tensor_scalar(
                                    out=th[:], in0=th[:], scalar1=TH_FLOOR,
                                    scalar2=None, op0=Alu.max,
                                )
                                thetas[i] = th
                                ab = pa.tile(
                                    [128, 512 * NSLOT], bf16, tag="ab", name="ab"
                                )
                                nc.vector.tensor_scalar(
                                    out=ab[:, :W], in0=sb[:], scalar1=th[:],
                                    scalar2=None, op0=Alu.is_ge,
                                )
                                k = slots.index(i) if hasattr(slots, "index") else list(slots).index(i)
                                at = pat.tile(
                                    [128, 4 * (i + 1) * 128], bf16,
                                    tag=f"at{k}", name=f"at{i}",
                                )
                                for q in range(4 * (i + 1)):
                                    nc.sync.dma_start(
                                        at[:, 128 * q : 128 * (q + 1)],
                                        ab[:, 128 * q : 128 * (q + 1)],
                                        transpose=True,
                                    )
                                ats[i] = at

                # ---------- msg + epilogue ----------
                with ExitStack() as mctx:
                    pmsg = mctx.enter_context(tc.tile_pool(name=f"pmsg{gidx}", bufs=1))
                    patf = mctx.enter_context(tc.tile_pool(name=f"patf{gidx}", bufs=1))
                    pxnc = mctx.enter_context(tc.tile_pool(name=f"pxnc{gidx}", bufs=2))
                    pepi = mctx.enter_context(tc.tile_pool(name=f"pepi{gidx}", bufs=2))
                    ppsm = mctx.enter_context(
                        tc.tile_pool(name=f"ppsm{gidx}", bufs=1, space="PSUM")
                    )
                    msgs = {}
                    atfs = {}
                    for k, i in enumerate(slots):
                        msgs[i] = pmsg.tile([128, D], f32, tag=f"msg{k}", name=f"msg{i}")
                        atf = patf.tile(
                            [128, 4 * (i + 1) * 128], f32, tag=f"atf{k}", name=f"atf{i}"
                        )
                        nc.vector.tensor_copy(atf[:], ats[i][:])
                        atfs[i] = atf
                    for h in range(2):
                        psms = {}
                        for k, i in enumerate(slots):
                            psms[i] = ppsm.tile(
                                [128, DH], f32, tag=f"psm{k}", name=f"psm{i}"
                            )
                        for c in range(gmax + 1):
                            xnc = pxnc.tile([128, 4, DH], f32, tag="xnc", name="xnc")
                            nc.sync.dma_start(
                                xnc[:],
                                xnat_v[:, 4 * c : 4 * (c + 1), DH * h : DH * (h + 1)],
                            )
                            for i in slots:
                                if i < c:
                                    continue
                                for sub in range(4):
                                    q = 4 * c + sub
                                    nc.tensor.matmul(
                                        psms[i][:],
                                        atfs[i][:, 128 * q : 128 * (q + 1)],
                                        xnc[:, sub, :],
                                        start=(q == 0), stop=(q == 4 * (i + 1) - 1),
                                    )
                        for i in slots:
                            # msg*(1-mix)*gain/deg: TT with g2 (PSUM src), then 1/deg
                            mseg = msgs[i][:, DH * h : DH * (h + 1)]
                            nc.vector.tensor_tensor(
                                out=mseg, in0=psms[i][:],
                                in1=t_g2[:, DH * h : DH * (h + 1)], op=Alu.mult,
                            )
                            nc.vector.tensor_scalar(
                                out=mseg, in0=mseg, scalar1=rdegs[i][:], scalar2=None,
                                op0=Alu.mult,
                            )
                    for i in slots:
                        xe = pepi.tile([128, D], f32, tag="xe", name="xe")
                        nc.sync.dma_start(xe[:], xrn[128 * i : 128 * (i + 1), :])
                        e1 = pepi.tile([128, D], f32, tag="e1", name="e1")
                        nc.vector.tensor_tensor(
                            out=e1[:], in0=xe[:], in1=t_g1[:], op=Alu.mult
                        )
                        nc.vector.tensor_tensor(
                            out=e1[:], in0=e1[:], in1=msgs[i][:], op=Alu.add
                        )
                        nc.vector.tensor_tensor(
                            out=e1[:], in0=e1[:], in1=t_bias[:], op=Alu.add
                        )
                        g = pepi.tile([128, D], f32, tag="g", name="g")
                        nc.scalar.activation(g[:], e1[:], gelu_func)
                        nc.vector.tensor_scalar(
                            out=g[:], in0=g[:], scalar1=t_sc[:, 1:2], scalar2=None,
                            op0=Alu.mult,
                        )
                        nc.sync.dma_start(out[128 * i : 128 * (i + 1), :], g[:])

    nc.compile()
    return nc


# ----------------------------------------------------------------------------
# Host-side sharding
# ----------------------------------------------------------------------------
def _softplus32(v):
    v = np.float32(v)
    return np.float32(np.log1p(np.exp(np.float64(v))))


def make_core_inputs(inputs, T=4096, D=1024):
    """Build the 8 per-core input maps from the full problem inputs."""
    x = np.ascontiguousarray(np.asarray(inputs["x"], dtype=np.float32))
    B = x.shape[0]
    NSLOT = T // 512
    R = NSLOT * 128
    f = np.float32

    mix = f(1.0 / (1.0 + np.exp(-np.float64(np.asarray(inputs["log_mix"])))))
    scale = _softplus32(np.asarray(inputs["log_scale"])) + f(0.01)
    sigma = _softplus32(np.asarray(inputs["log_sigma_raw"])) + f(0.01)
    ema_mean = np.asarray(inputs["ema_mean"], dtype=np.float32)
    ema_sq = np.asarray(inputs["ema_sq"], dtype=np.float32)
    gain = np.asarray(inputs["gain"], dtype=np.float32)
    bias = np.asarray(inputs["bias"], dtype=np.float32)

    std = np.sqrt(np.clip(ema_sq - ema_mean * ema_mean, f(1e-6), None)).astype(f)
    inv_std = (f(1.0) / std).astype(f)
    negm = (-ema_mean * inv_std).astype(f)
    g1 = (gain * mix).astype(f)
    g2 = (gain * (f(1.0) - mix)).astype(f)

    def rep(v):
        return np.ascontiguousarray(np.tile(v[None, :], (128, 1)).astype(f))

    iota16 = np.ascontiguousarray(
        np.tile(np.arange(16, dtype=f)[None, :], (128, 1))
    )

    in_maps = []
    rows_by_core = []
    for c in range(8):
        b, j = c // 4, c % 4
        rows = np.concatenate(
            [512 * i + j + 4 * np.arange(128) for i in range(NSLOT)]
        ).astype(np.int64)
        rows_by_core.append((b, rows))
        xb = x[b]
        m = np.arange(128)
        dmask = np.where(
            np.arange(512)[None, :] < (j + 4 * m)[:, None], f(0.0), f(NEG_BIG)
        ).astype(f)
        trow = np.stack(
            [(512 * i + j + 4 * m).astype(f) for i in range(NSLOT)], axis=1
        )
        sc = np.zeros((128, 2), f)
        sc[:, 0] = sigma / f(D)
        sc[:, 1] = scale
        in_maps.append(
            {
                "xT": np.ascontiguousarray(xb.T),
                "xrT": np.ascontiguousarray(xb[rows].T),
                "xnat": xb,
                "xrn": np.ascontiguousarray(xb[rows]),
                "invstd_b": rep(inv_std),
                "negm_b": rep(negm),
                "g1_b": rep(g1),
                "g2_b": rep(g2),
                "bias_b": rep(bias),
                "dmask": np.ascontiguousarray(dmask),
                "iota16": iota16,
                "trow": np.ascontiguousarray(trow),
                "sc": sc,
            }
        )
    return in_maps, rows_by_core, (B, x.shape[1], x.shape[2])


_PROG_CACHE = {}


def kernel(**inputs):
    from concourse.bass_utils import run_bass_kernel_spmd

    x = np.asarray(inputs["x"])
    B, T, D = x.shape
    key = (T, D)
    if key not in _PROG_CACHE:
        _PROG_CACHE[key] = build_program(T=T, D=D)
    nc = _PROG_CACHE[key]
    in_maps, rows_by_core, _ = make_core_inputs(inputs, T=T, D=D)
    res = run_bass_kernel_spmd(nc, in_maps, list(range(8)))
    y = np.zeros((B, T, D), dtype=np.float32)
    for c in range(8):
        b, rows = rows_by_core[c]
        y[b, rows, :] = res.results[c]["out"]
    return y


if __name__ == "__main__":
    rng = np.random.default_rng(0)
    B, T, D = 2, 4096, 1024
    demo = {
        "x": rng.standard_normal((B, T, D)).astype(np.float32),
        "gain": np.ones(D, np.float32),
        "bias": np.zeros(D, np.float32),
        "log_mix": np.zeros((), np.float32),
        "log_scale": np.zeros((), np.float32),
        "log_sigma_raw": np.zeros((), np.float32),
        "logit_decay": np.zeros((), np.float32),
        "ema_mean": np.zeros(D, np.float32),
        "ema_sq": np.ones(D, np.float32),
    }
    y = kernel(**demo)
    print("out", y.shape, float(np.abs(y).max()))



# revision 32
# speedup vs baseline: 1.1073x; 1.1073x over previous
"""Trainium2 Bass kernel for nn_DGN2_70428873720402 (gnn_message_passing).

Math (per batch b):
  surp_t  = tanh(sigma * mean_d |(x - ema_mean)/std|)           (B,T)
  K_t     = clip(round(2 + 14*surp), 0, 16)
  sim     = cosine-similarity(x_t, x_s), strictly causal (s < t)
  A[t,s]  = 1 iff sim[t,s] among top-K_t of row t               (threshold form)
  msg     = (A @ x) / max(min(K_t, t), 1)
  out     = gelu((mix*x + (1-mix)*msg)*gain + bias) * scale

Sharding: 8 cores = 2 batches x 4 row-stripes.  Core (b, j) owns global rows
t = 512*i + j + 4*m  (slot i in 0..7, m in 0..127) - every core has the same
slot widths W_i = 512*(i+1), so one SPMD program serves all 8 cores; the
stripe offset j only enters through host-prepared data.

Ranking key: sim'[t,s] = x_t . xn_s with xn = x/||x|| (column-normalized; the
1/||x_t|| row factor cannot change a within-row ranking).  The dot products
run on the PE in bf16 via a hi/lo split (x = hi + lo, each bf16):
  sim' = hi_l.hi_r + hi_l.lo_r + lo_l.hi_r     (drops only lo.lo ~ 2^-18)
which preserves fp32-level ranking fidelity at 3/4 the PE cost of native
fp32 (which runs 4 passes/row).  Per-row top-K via the DVE Max8 ISA:
max -> match_replace -> max gives the sorted top-16; the K_t-th value
(selected with an iota mask) is the threshold; A = (sim >= theta) & causal,
stored bf16 per slot.  theta is floored at -1e8 so rows with t <= K_t select
exactly their t causal predecessors (mask slots = -1e9).

Engine split: PE does sim/transpose/msg matmuls back to back (A^T transposes
are deferred to the aggregation phase so the PE never waits on the DVE topk
chain); scalar does PSUM evacuation + activations; DVE does topk/threshold;
gpsimd does stats/epilogue elementwise and A^T PSUM evacuation.
"""

import sys

sys.path.insert(0, "/opt/trn_rl_repo")

import numpy as np
from contextlib import ExitStack

import concourse.bass as bass
import concourse.bacc as bacc
import concourse.mybir as mybir
import concourse.tile as tile

f32 = mybir.dt.float32
bf16 = mybir.dt.bfloat16
Alu = mybir.AluOpType
Act = mybir.ActivationFunctionType

NEG_BIG = -1.0e9
TH_FLOOR = -1.0e8
K_HIGH, K_LOW = 16, 2


def build_program(T=4096, D=1024, groups=((0, 1, 2, 3), (4, 5, 6, 7)), gelu_func=None):
    if gelu_func is None:
        gelu_func = Act.Gelu
    NSLOT = T // 512
    KD = D // 128
    R = NSLOT * 128
    HC = 512
    DH = D // 2

    nc = bacc.Bacc("TRN2", target_bir_lowering=False, debug=False)

    xnT_hi = nc.dram_tensor("xnT_hi", [D, T], bf16, kind="ExternalInput")
    xnT_lo = nc.dram_tensor("xnT_lo", [D, T], bf16, kind="ExternalInput")
    xrT_hi = nc.dram_tensor("xrT_hi", [D, R], bf16, kind="ExternalInput")
    xrT_lo = nc.dram_tensor("xrT_lo", [D, R], bf16, kind="ExternalInput")
    xnat = nc.dram_tensor("xnat", [T, D], bf16, kind="ExternalInput")
    xrn = nc.dram_tensor("xrn", [R, D], f32, kind="ExternalInput")
    invstd_b = nc.dram_tensor("invstd_b", [128, D], f32, kind="ExternalInput")
    negm_b = nc.dram_tensor("negm_b", [128, D], f32, kind="ExternalInput")
    g1_b = nc.dram_tensor("g1_b", [128, D], f32, kind="ExternalInput")
    g2_b = nc.dram_tensor("g2_b", [128, D], f32, kind="ExternalInput")
    bias_b = nc.dram_tensor("bias_b", [128, D], f32, kind="ExternalInput")
    dmask = nc.dram_tensor("dmask", [128, 512], f32, kind="ExternalInput")
    iota16 = nc.dram_tensor("iota16", [128, 16], f32, kind="ExternalInput")
    trow = nc.dram_tensor("trow", [128, NSLOT], f32, kind="ExternalInput")
    sc = nc.dram_tensor("sc", [128, 2], f32, kind="ExternalInput")
    ident = nc.dram_tensor("ident", [128, 128], bf16, kind="ExternalInput")
    out = nc.dram_tensor("out", [R, D], f32, kind="ExternalOutput")

    xnT_hi_v = xnT_hi.rearrange("(k p) t -> p k t", p=128)
    xnT_lo_v = xnT_lo.rearrange("(k p) t -> p k t", p=128)
    xrT_hi_v = xrT_hi.rearrange("(k p) m -> p k m", p=128)
    xrT_lo_v = xrT_lo.rearrange("(k p) m -> p k m", p=128)
    xnat_v = xnat.rearrange("(a p) d -> p a d", p=128)

    with tile.TileContext(nc) as tc, ExitStack() as ctx:
        prm = ctx.enter_context(tc.tile_pool(name="prm", bufs=1))
        ptiny = ctx.enter_context(tc.tile_pool(name="ptiny", bufs=1))
        pab = ctx.enter_context(tc.tile_pool(name="pab", bufs=1))
        # stream + lhs pools live for the whole kernel and rotate buffers, so
        # each group's loads take WAR deps on the previous group's
        # (early-freed) sim reads instead of on late epilogue readers.
        pstream = ctx.enter_context(tc.tile_pool(name="pstream", bufs=2))
        plhs = ctx.enter_context(tc.tile_pool(name="plhs", bufs=2))

        # ---- global params ----
        t_g1 = prm.tile([128, D], f32, tag="g1")
        nc.scalar.dma_start(t_g1[:], g1_b[:])
        t_g2 = prm.tile([128, D], f32, tag="g2")
        nc.scalar.dma_start(t_g2[:], g2_b[:])
        t_bias = prm.tile([128, D], f32, tag="bias")
        nc.scalar.dma_start(t_bias[:], bias_b[:])
        t_dmask = prm.tile([128, 512], f32, tag="dmask")
        nc.scalar.dma_start(t_dmask[:], dmask[:])
        t_iota = prm.tile([128, 16], f32, tag="iota")
        nc.scalar.dma_start(t_iota[:], iota16[:])
        t_trow = prm.tile([128, NSLOT], f32, tag="trow")
        nc.scalar.dma_start(t_trow[:], trow[:])
        t_sc = prm.tile([128, 2], f32, tag="sc")
        nc.scalar.dma_start(t_sc[:], sc[:])
        t_ident = prm.tile([128, 128], bf16, tag="ident")
        nc.scalar.dma_start(t_ident[:], ident[:])

        # ---- phase 1: per-slot stats -> kmask, rdeg ----
        # z-prep runs on gpsimd; scalar does Abs x8 then Tanh x8 (2 act
        # table loads); DVE handles the tiny kmask math.
        kmasks, rdegs, sabss = {}, {}, {}
        if True:
            pstat = ctx.enter_context(tc.tile_pool(name="pstat", bufs=1))
            t_invstd = pstat.tile([128, D], f32, tag="invstd", bufs=1)
            nc.scalar.dma_start(t_invstd[:], invstd_b[:])
            t_negm = pstat.tile([128, D], f32, tag="negm", bufs=1)
            nc.scalar.dma_start(t_negm[:], negm_b[:])
            for i in range(NSLOT):
                xs = pstat.tile([128, D], f32, tag="xs")
                nc.scalar.dma_start(xs[:], xrn[128 * i : 128 * (i + 1), :])
                z = pstat.tile([128, D], f32, tag="z")
                nc.gpsimd.tensor_tensor(out=z[:], in0=xs[:], in1=t_invstd[:], op=Alu.mult)
                nc.gpsimd.tensor_tensor(out=z[:], in0=z[:], in1=t_negm[:], op=Alu.add)
                sabs = ptiny.tile([128, 1], f32, tag=f"sabs{i}", name=f"sabs{i}")
                nc.scalar.activation(z[:], z[:], Act.Abs, accum_out=sabs[:])
                sabss[i] = sabs
            for i in range(NSLOT):
                surp = ptiny.tile([128, 1], f32, tag=f"surp{i}", name=f"surp{i}")
                nc.scalar.activation(surp[:], sabss[i][:], Act.Tanh, scale=t_sc[:, 0:1])
                kraw = ptiny.tile([128, 1], f32, tag=f"kraw{i}", name=f"kraw{i}")
                nc.vector.tensor_scalar(
                    out=kraw[:], in0=surp[:], scalar1=float(K_HIGH - K_LOW),
                    scalar2=float(K_LOW), op0=Alu.mult, op1=Alu.add,
                )
                km05 = ptiny.tile([128, 1], f32, tag=f"km05_{i}", name=f"km05_{i}")
                nc.vector.tensor_scalar(
                    out=km05[:], in0=kraw[:], scalar1=0.5, scalar2=None, op0=Alu.subtract
                )
                km15 = ptiny.tile([128, 1], f32, tag=f"km15_{i}", name=f"km15_{i}")
                nc.vector.tensor_scalar(
                    out=km15[:], in0=kraw[:], scalar1=1.5, scalar2=None, op0=Alu.subtract
                )
                m1 = ptiny.tile([128, 16], f32, tag=f"m1_{i}", name=f"m1_{i}")
                nc.vector.tensor_scalar(
                    out=m1[:], in0=t_iota[:], scalar1=km05[:], scalar2=None, op0=Alu.is_le
                )
                m2 = ptiny.tile([128, 16], f32, tag=f"m2_{i}", name=f"m2_{i}")
                nc.vector.tensor_scalar(
                    out=m2[:], in0=t_iota[:], scalar1=km15[:], scalar2=None, op0=Alu.is_gt
                )
                kmask = ptiny.tile([128, 16], f32, tag=f"kmask{i}", name=f"kmask{i}")
                nc.vector.tensor_tensor(out=kmask[:], in0=m1[:], in1=m2[:], op=Alu.mult)
                kmasks[i] = kmask
                kss = ptiny.tile([128, 16], f32, tag=f"kss{i}", name=f"kss{i}")
                km1 = ptiny.tile([128, 1], f32, tag=f"km1_{i}", name=f"km1_{i}")
                nc.vector.tensor_tensor(
                    out=kss[:], in0=kmask[:], in1=t_iota[:], op=Alu.mult
                )
                nc.vector.tensor_reduce(
                    out=km1[:], in_=kss[:], axis=mybir.AxisListType.X, op=Alu.add
                )
                deg = ptiny.tile([128, 1], f32, tag=f"deg{i}", name=f"deg{i}")
                nc.vector.tensor_scalar(
                    out=deg[:], in0=km1[:], scalar1=1.0, scalar2=t_trow[:, i : i + 1],
                    op0=Alu.add, op1=Alu.min,
                )
                nc.vector.tensor_scalar(
                    out=deg[:], in0=deg[:], scalar1=1.0, scalar2=None, op0=Alu.max
                )
                rdeg = ptiny.tile([128, 1], f32, tag=f"rdeg{i}", name=f"rdeg{i}")
                nc.vector.reciprocal(rdeg[:], deg[:])
                rdegs[i] = rdeg

        # ---- phase 2: sim + topk -> per-slot A (row-major bf16) ----
        abs_ = {}
        for i in range(NSLOT):
            abs_[i] = pab.tile(
                [128, 512 * (i + 1)], bf16, tag=f"ab{i}", name=f"ab{i}"
            )
        for gidx, slots in enumerate(groups):
            gmax = max(slots)
            n_hc = gmax + 1
            with ExitStack() as sctx:
                psim = sctx.enter_context(tc.tile_pool(name=f"psim{gidx}", bufs=1))
                prep = sctx.enter_context(tc.tile_pool(name=f"prep{gidx}", bufs=1))
                pps = sctx.enter_context(
                    tc.tile_pool(name=f"pps{gidx}", bufs=3, space="PSUM")
                )
                g0r = 128 * list(slots)[0]
                lhg_hi = plhs.tile([128, KD, 512], bf16, tag="lhgh", name="lhgh")
                nc.sync.dma_start(lhg_hi[:], xrT_hi_v[:, :, g0r : g0r + 512])
                lhg_lo = plhs.tile([128, KD, 512], bf16, tag="lhgl", name="lhgl")
                nc.sync.dma_start(lhg_lo[:], xrT_lo_v[:, :, g0r : g0r + 512])
                lhs_hi, lhs_lo = {}, {}
                for k, i in enumerate(slots):
                    lhs_hi[i] = [
                        lhg_hi[:, dk, 128 * k : 128 * (k + 1)] for dk in range(KD)
                    ]
                    lhs_lo[i] = [
                        lhg_lo[:, dk, 128 * k : 128 * (k + 1)] for dk in range(KD)
                    ]
                simbufs = {}
                for k, i in enumerate(slots):
                    simbufs[i] = psim.tile(
                        [128, 512 * (i + 1)], f32, tag=f"sim{k}", name=f"sim{i}"
                    )
                for hc in range(n_hc):
                    c0 = HC * hc
                    rh = pstream.tile([128, KD, HC], bf16, tag="rsh", name="rsh")
                    nc.sync.dma_start(rh[:], xnT_hi_v[:, :, c0 : c0 + HC])
                    rl = pstream.tile([128, KD, HC], bf16, tag="rsl", name="rsl")
                    nc.sync.dma_start(rl[:], xnT_lo_v[:, :, c0 : c0 + HC])
                    for i in slots:
                        if i < hc:
                            continue
                        psg = pps.tile([128, HC], f32, tag="psg")
                        for dk in range(KD):
                            nc.tensor.matmul(
                                psg[:], lhs_hi[i][dk], rh[:, dk, :],
                                start=(dk == 0), stop=False,
                            )
                            nc.tensor.matmul(
                                psg[:], lhs_hi[i][dk], rl[:, dk, :],
                                start=False, stop=False,
                            )
                            nc.tensor.matmul(
                                psg[:], lhs_lo[i][dk], rh[:, dk, :],
                                start=False, stop=(dk == KD - 1),
                            )
                        seg = simbufs[i][:, c0 : c0 + HC]
                        if hc != i:
                            nc.scalar.activation(seg, psg[:], Act.Copy)
                            continue
                        # diagonal chunk: fuse the causal staircase mask,
                        # then run the topk/threshold chain on the DVE (the
                        # PE proceeds with the other slots' matmuls).
                        nc.vector.tensor_tensor(
                            out=seg, in0=psg[:], in1=t_dmask[:], op=Alu.add
                        )
                        W = 512 * (i + 1)
                        sb = simbufs[i]
                        top16 = ptiny.tile(
                            [128, 16], f32, tag=f"top16_{i}", name=f"top16_{i}"
                        )
                        nc.vector.max(top16[:, 0:8], sb[:])
                        rep = prep.tile(
                            [128, 512 * NSLOT], f32, tag="rep", name="rep"
                        )
                        nc.vector.match_replace(
                            rep[:, :W], top16[:, 0:8], sb[:], NEG_BIG
                        )
                        nc.vector.max(top16[:, 8:16], rep[:, :W])
                        tts = ptiny.tile([128, 16], f32, tag=f"tts{i}", name=f"tts{i}")
                        th = ptiny.tile([128, 1], f32, tag=f"th{i}", name=f"th{i}")
                        nc.vector.tensor_tensor(
                            out=tts[:], in0=top16[:], in1=kmasks[i][:], op=Alu.mult
                        )
                        nc.vector.tensor_reduce(
                            out=th[:], in_=tts[:], axis=mybir.AxisListType.X, op=Alu.add
                        )
                        nc.vector.tensor_scalar(
                            out=th[:], in0=th[:], scalar1=TH_FLOOR, scalar2=None,
                            op0=Alu.max,
                        )
                        nc.vector.tensor_scalar(
                            out=abs_[i][:], in0=sb[:], scalar1=th[:], scalar2=None,
                            op0=Alu.is_ge,
                        )

            # ---- A^T (PE transpose) + aggregation + epilogue (this group) ----
            # Two sub-passes (low slots, then high slots): the high slots'
            # topk chains still drain on the DVE while the low slots'
            # transposes + matmuls keep the PE fed.
            sidx = gidx
            with ExitStack() as mctx:
                pmsg = mctx.enter_context(tc.tile_pool(name=f"pmsg{sidx}", bufs=1))
                pxnc = mctx.enter_context(tc.tile_pool(name=f"pxnc{sidx}", bufs=3))
                pepi = mctx.enter_context(tc.tile_pool(name=f"pepi{sidx}", bufs=1))
                pat = mctx.enter_context(tc.tile_pool(name=f"pat{sidx}", bufs=1))
                ppsm = mctx.enter_context(
                    tc.tile_pool(name=f"ppsm{sidx}", bufs=1, space="PSUM")
                )
                ptr = mctx.enter_context(
                    tc.tile_pool(name=f"ptr{sidx}", bufs=2, space="PSUM")
                )
                ats, msgs, xes = {}, {}, {}
                for k, i in enumerate(slots):
                    ats[i] = pat.tile(
                        [128, 4 * (i + 1) * 128], bf16, tag=f"at{k}", name=f"at{i}"
                    )
                    msgs[i] = pmsg.tile([128, D], f32, tag=f"msg{k}", name=f"msg{i}")
                    xe = pepi.tile([128, D], f32, tag="xe", name=f"xe{i}", bufs=2)
                    nc.gpsimd.dma_start(xe[:], xrn[128 * i : 128 * (i + 1), :])
                    xes[i] = xe
                # batched A^T: PE transposes run back-to-back; scalar/DVE
                # copies drain behind them without stalling the PE queue
                for i in slots:
                    for q in range(4 * (i + 1)):
                        pt = ptr.tile([128, 128], bf16, tag="pt")
                        nc.tensor.transpose(
                            pt[:], abs_[i][:, 128 * q : 128 * (q + 1)], t_ident[:]
                        )
                        if q % 2 == 0:
                            nc.scalar.activation(
                                ats[i][:, 128 * q : 128 * (q + 1)], pt[:], Act.Copy
                            )
                        else:
                            nc.vector.tensor_copy(
                                ats[i][:, 128 * q : 128 * (q + 1)], pt[:]
                            )
                for h in range(2):
                    psms = {}
                    for k, i in enumerate(slots):
                        psms[i] = ppsm.tile(
                            [128, DH], f32, tag=f"psm{k}", name=f"psm{i}"
                        )
                    for c in range(gmax + 1):
                        xnc = pxnc.tile([128, 4, DH], bf16, tag="xnc", name="xnc")
                        nc.gpsimd.dma_start(
                            xnc[:],
                            xnat_v[:, 4 * c : 4 * (c + 1), DH * h : DH * (h + 1)],
                        )
                        for i in slots:
                            if i < c:
                                continue
                            for sub in range(4):
                                q = 4 * c + sub
                                nc.tensor.matmul(
                                    psms[i][:],
                                    ats[i][:, 128 * q : 128 * (q + 1)],
                                    xnc[:, sub, :],
                                    start=(q == 0), stop=(q == 4 * (i + 1) - 1),
                                )
                            if i != c:
                                continue
                            # slot done accumulating: evacuate + scale now
                            # msg*(1-mix)*gain/deg: TT with g2 (PSUM src), 1/deg
                            mseg = msgs[i][:, DH * h : DH * (h + 1)]
                            nc.vector.tensor_tensor(
                                out=mseg, in0=psms[i][:],
                                in1=t_g2[:, DH * h : DH * (h + 1)], op=Alu.mult,
                            )
                            nc.vector.tensor_scalar(
                                out=mseg, in0=mseg, scalar1=rdegs[i][:],
                                scalar2=None, op0=Alu.mult,
                            )
                            if h != 1:
                                continue
                            # both halves done: epilogue for this slot
                            e1 = pepi.tile([128, D], f32, tag="e1", name="e1", bufs=1)
                            nc.vector.tensor_tensor(
                                out=e1[:], in0=xes[i][:], in1=t_g1[:], op=Alu.mult
                            )
                            nc.vector.tensor_tensor(
                                out=e1[:], in0=e1[:], in1=msgs[i][:], op=Alu.add
                            )
                            nc.vector.tensor_tensor(
                                out=e1[:], in0=e1[:], in1=t_bias[:], op=Alu.add
                            )
                            g = pepi.tile([128, D], f32, tag="g", name="g", bufs=1)
                            nc.scalar.activation(g[:], e1[:], gelu_func)
                            nc.vector.tensor_scalar(
                                out=g[:], in0=g[:], scalar1=t_sc[:, 1:2],
                                scalar2=None, op0=Alu.mult,
                            )
                            nc.scalar.dma_start(out[128 * i : 128 * (i + 1), :], g[:])

    nc.compile()
    return nc


# ----------------------------------------------------------------------------
# Host-side sharding
# ----------------------------------------------------------------------------
def _softplus32(v):
    v = np.float32(v)
    return np.float32(np.log1p(np.exp(np.float64(v))))


def make_core_inputs(inputs, T=4096, D=1024):
    """Build the 8 per-core input maps from the full problem inputs."""
    import ml_dtypes

    bf = ml_dtypes.bfloat16
    x = np.ascontiguousarray(np.asarray(inputs["x"], dtype=np.float32))
    B = x.shape[0]
    NSLOT = T // 512
    R = NSLOT * 128
    f = np.float32

    mix = f(1.0 / (1.0 + np.exp(-np.float64(np.asarray(inputs["log_mix"])))))
    scale = _softplus32(np.asarray(inputs["log_scale"])) + f(0.01)
    sigma = _softplus32(np.asarray(inputs["log_sigma_raw"])) + f(0.01)
    ema_mean = np.asarray(inputs["ema_mean"], dtype=np.float32)
    ema_sq = np.asarray(inputs["ema_sq"], dtype=np.float32)
    gain = np.asarray(inputs["gain"], dtype=np.float32)
    bias = np.asarray(inputs["bias"], dtype=np.float32)

    std = np.sqrt(np.clip(ema_sq - ema_mean * ema_mean, f(1e-6), None)).astype(f)
    inv_std = (f(1.0) / std).astype(f)
    negm = (-ema_mean * inv_std).astype(f)
    g1 = (gain * mix).astype(f)
    g2 = (gain * (f(1.0) - mix)).astype(f)

    def rep(v):
        return np.ascontiguousarray(np.tile(v[None, :], (128, 1)).astype(f))

    iota16 = np.ascontiguousarray(
        np.tile(np.arange(16, dtype=f)[None, :], (128, 1))
    )
    ident = np.ascontiguousarray(np.eye(128, dtype=bf))

    # per-batch: normalized columns (hi/lo bf16) + raw bf16 rows for aggregation
    b_xnT_hi, b_xnT_lo, b_xnat = [], [], []
    for b in range(B):
        xb = x[b]
        n64 = np.linalg.norm(xb.astype(np.float64), axis=1)
        n64 = np.maximum(n64, 1e-12)
        xn = (xb.astype(np.float64) / n64[:, None]).astype(f)
        xn_hi = xn.astype(bf)
        xn_lo = (xn - xn_hi.astype(f)).astype(bf)
        b_xnT_hi.append(np.ascontiguousarray(xn_hi.T))
        b_xnT_lo.append(np.ascontiguousarray(xn_lo.T))
        b_xnat.append(np.ascontiguousarray(xb.astype(bf)))

    in_maps = []
    rows_by_core = []
    for c in range(8):
        b, j = c // 4, c % 4
        rows = np.concatenate(
            [512 * i + j + 4 * np.arange(128) for i in range(NSLOT)]
        ).astype(np.int64)
        rows_by_core.append((b, rows))
        xb = x[b]
        xr = np.ascontiguousarray(xb[rows])
        xr_hi = xr.astype(bf)
        xr_lo = (xr - xr_hi.astype(f)).astype(bf)
        m = np.arange(128)
        dmask = np.where(
            np.arange(512)[None, :] < (j + 4 * m)[:, None], f(0.0), f(NEG_BIG)
        ).astype(f)
        trow = np.stack(
            [(512 * i + j + 4 * m).astype(f) for i in range(NSLOT)], axis=1
        )
        sc = np.zeros((128, 2), f)
        sc[:, 0] = sigma / f(D)
        sc[:, 1] = scale
        in_maps.append(
            {
                "xnT_hi": b_xnT_hi[b],
                "xnT_lo": b_xnT_lo[b],
                "xrT_hi": np.ascontiguousarray(xr_hi.T),
                "xrT_lo": np.ascontiguousarray(xr_lo.T),
                "xnat": b_xnat[b],
                "xrn": xr,
                "invstd_b": rep(inv_std),
                "negm_b": rep(negm),
                "g1_b": rep(g1),
                "g2_b": rep(g2),
                "bias_b": rep(bias),
                "dmask": np.ascontiguousarray(dmask),
                "iota16": iota16,
                "trow": np.ascontiguousarray(trow),
                "sc": sc,
                "ident": ident,
            }
        )
    return in_maps, rows_by_core, (B, x.shape[1], x.shape[2])


_PROG_CACHE = {}


def kernel(**inputs):
    from concourse.bass_utils import run_bass_kernel_spmd

    x = np.asarray(inputs["x"])
    B, T, D = x.shape
    key = (T, D)
    if key not in _PROG_CACHE:
        _PROG_CACHE[key] = build_program(T=T, D=D)
    nc = _PROG_CACHE[key]
    in_maps, rows_by_core, _ = make_core_inputs(inputs, T=T, D=D)
    res = run_bass_kernel_spmd(nc, in_maps, list(range(8)))
    y = np.zeros((B, T, D), dtype=np.float32)
    for c in range(8):
        b, rows = rows_by_core[c]
        y[b, rows, :] = res.results[c]["out"]
    return y


if __name__ == "__main__":
    rng = np.random.default_rng(0)
    B, T, D = 2, 4096, 1024
    demo = {
        "x": rng.standard_normal((B, T, D)).astype(np.float32),
        "gain": np.ones(D, np.float32),
        "bias": np.zeros(D, np.float32),
        "log_mix": np.zeros((), np.float32),
        "log_scale": np.zeros((), np.float32),
        "log_sigma_raw": np.zeros((), np.float32),
        "logit_decay": np.zeros((), np.float32),
        "ema_mean": np.zeros(D, np.float32),
        "ema_sq": np.ones(D, np.float32),
    }
    y = kernel(**demo)
    print("out", y.shape, float(np.abs(y).max()))


# revision 33
# speedup vs baseline: 1.1351x; 1.0251x over previous
"""Trainium2 Bass kernel for nn_DGN2_70428873720402 (gnn_message_passing).

Math (per batch b):
  surp_t  = tanh(sigma * mean_d |(x - ema_mean)/std|)           (B,T)
  K_t     = clip(round(2 + 14*surp), 0, 16)
  sim     = cosine-similarity(x_t, x_s), strictly causal (s < t)
  A[t,s]  = 1 iff sim[t,s] among top-K_t of row t               (threshold form)
  msg     = (A @ x) / max(min(K_t, t), 1)
  out     = gelu((mix*x + (1-mix)*msg)*gain + bias) * scale

Sharding: 8 cores = 2 batches x 4 row-stripes.  Core (b, j) owns global rows
t = 512*i + j + 4*m  (slot i in 0..7, m in 0..127) - every core has the same
slot widths W_i = 512*(i+1), so one SPMD program serves all 8 cores; the
stripe offset j only enters through host-prepared data.

Ranking key: sim'[t,s] = x_t . xn_s with xn = x/||x|| (column-normalized; the
1/||x_t|| row factor cannot change a within-row ranking).  The dot products
run on the PE in bf16 via a hi/lo split (x = hi + lo, each bf16):
  sim' = hi_l.hi_r + hi_l.lo_r + lo_l.hi_r     (drops only lo.lo ~ 2^-18)
which preserves fp32-level ranking fidelity at 3/4 the PE cost of native
fp32 (which runs 4 passes/row).  Per-row top-K via the DVE Max8 ISA:
max -> match_replace -> max gives the sorted top-16; the K_t-th value
(selected with an iota mask) is the threshold; A = (sim >= theta) & causal,
stored bf16 per slot.  theta is floored at -1e8 so rows with t <= K_t select
exactly their t causal predecessors (mask slots = -1e9).

Engine split: PE does sim/transpose/msg matmuls back to back (A^T transposes
are deferred to the aggregation phase so the PE never waits on the DVE topk
chain); scalar does PSUM evacuation + activations; DVE does topk/threshold;
gpsimd does stats/epilogue elementwise and A^T PSUM evacuation.
"""

import sys

sys.path.insert(0, "/opt/trn_rl_repo")

import numpy as np
from contextlib import ExitStack

import concourse.bass as bass
import concourse.bacc as bacc
import concourse.mybir as mybir
import concourse.tile as tile

f32 = mybir.dt.float32
bf16 = mybir.dt.bfloat16
Alu = mybir.AluOpType
Act = mybir.ActivationFunctionType

NEG_BIG = -1.0e9
TH_FLOOR = -1.0e8
K_HIGH, K_LOW = 16, 2


def build_program(T=4096, D=1024, groups=((0, 1, 2, 3), (4, 5, 6, 7)), gelu_func=None):
    if gelu_func is None:
        gelu_func = Act.Gelu
    NSLOT = T // 512
    KD = D // 128
    R = NSLOT * 128
    HC = 512
    DH = D // 2

    nc = bacc.Bacc("TRN2", target_bir_lowering=False, debug=False)

    xnT_hi = nc.dram_tensor("xnT_hi", [D, T], bf16, kind="ExternalInput")
    xnT_lo = nc.dram_tensor("xnT_lo", [D, T], bf16, kind="ExternalInput")
    xrT_hi = nc.dram_tensor("xrT_hi", [D, R], bf16, kind="ExternalInput")
    xrT_lo = nc.dram_tensor("xrT_lo", [D, R], bf16, kind="ExternalInput")
    xnat = nc.dram_tensor("xnat", [T, D], bf16, kind="ExternalInput")
    xrn = nc.dram_tensor("xrn", [R, D], f32, kind="ExternalInput")
    invstd_b = nc.dram_tensor("invstd_b", [128, D], f32, kind="ExternalInput")
    negm_b = nc.dram_tensor("negm_b", [128, D], f32, kind="ExternalInput")
    g1_b = nc.dram_tensor("g1_b", [128, D], f32, kind="ExternalInput")
    g2_b = nc.dram_tensor("g2_b", [128, D], f32, kind="ExternalInput")
    bias_b = nc.dram_tensor("bias_b", [128, D], f32, kind="ExternalInput")
    dmask = nc.dram_tensor("dmask", [128, 512], f32, kind="ExternalInput")
    iota16 = nc.dram_tensor("iota16", [128, 16], f32, kind="ExternalInput")
    trow = nc.dram_tensor("trow", [128, NSLOT], f32, kind="ExternalInput")
    sc = nc.dram_tensor("sc", [128, 2], f32, kind="ExternalInput")
    ident = nc.dram_tensor("ident", [128, 128], bf16, kind="ExternalInput")
    out = nc.dram_tensor("out", [R, D], f32, kind="ExternalOutput")

    xnT_hi_v = xnT_hi.rearrange("(k p) t -> p k t", p=128)
    xnT_lo_v = xnT_lo.rearrange("(k p) t -> p k t", p=128)
    xrT_hi_v = xrT_hi.rearrange("(k p) m -> p k m", p=128)
    xrT_lo_v = xrT_lo.rearrange("(k p) m -> p k m", p=128)
    xnat_v = xnat.rearrange("(a p) d -> p a d", p=128)

    with tile.TileContext(nc) as tc, ExitStack() as ctx:
        prm = ctx.enter_context(tc.tile_pool(name="prm", bufs=1))
        ptiny = ctx.enter_context(tc.tile_pool(name="ptiny", bufs=1))
        pab = ctx.enter_context(tc.tile_pool(name="pab", bufs=1))

        # ---- global params ----
        t_g1 = prm.tile([128, D], f32, tag="g1")
        nc.scalar.dma_start(t_g1[:], g1_b[:])
        t_g2 = prm.tile([128, D], f32, tag="g2")
        nc.scalar.dma_start(t_g2[:], g2_b[:])
        t_bias = prm.tile([128, D], f32, tag="bias")
        nc.scalar.dma_start(t_bias[:], bias_b[:])
        t_dmask = prm.tile([128, 512], f32, tag="dmask")
        nc.scalar.dma_start(t_dmask[:], dmask[:])
        t_iota = prm.tile([128, 16], f32, tag="iota")
        nc.scalar.dma_start(t_iota[:], iota16[:])
        t_trow = prm.tile([128, NSLOT], f32, tag="trow")
        nc.scalar.dma_start(t_trow[:], trow[:])
        t_sc = prm.tile([128, 2], f32, tag="sc")
        nc.scalar.dma_start(t_sc[:], sc[:])
        t_ident = prm.tile([128, 128], bf16, tag="ident")
        nc.scalar.dma_start(t_ident[:], ident[:])

        # ---- phase 1: per-slot stats -> kmask, rdeg ----
        # z-prep runs on gpsimd; scalar does Abs x8 then Tanh x8 (2 act
        # table loads); DVE handles the tiny kmask math.
        kmasks, rdegs, sabss = {}, {}, {}
        if True:
            pstat = ctx.enter_context(tc.tile_pool(name="pstat", bufs=1))
            t_invstd = pstat.tile([128, D], f32, tag="invstd", bufs=1)
            nc.scalar.dma_start(t_invstd[:], invstd_b[:])
            t_negm = pstat.tile([128, D], f32, tag="negm", bufs=1)
            nc.scalar.dma_start(t_negm[:], negm_b[:])
            for i in range(NSLOT):
                xs = pstat.tile([128, D], f32, tag="xs")
                nc.scalar.dma_start(xs[:], xrn[128 * i : 128 * (i + 1), :])
                z = pstat.tile([128, D], f32, tag="z")
                nc.gpsimd.tensor_tensor(out=z[:], in0=xs[:], in1=t_invstd[:], op=Alu.mult)
                nc.gpsimd.tensor_tensor(out=z[:], in0=z[:], in1=t_negm[:], op=Alu.add)
                sabs = ptiny.tile([128, 1], f32, tag=f"sabs{i}", name=f"sabs{i}")
                nc.scalar.activation(z[:], z[:], Act.Abs, accum_out=sabs[:])
                sabss[i] = sabs
            for i in range(NSLOT):
                surp = ptiny.tile([128, 1], f32, tag=f"surp{i}", name=f"surp{i}")
                nc.scalar.activation(surp[:], sabss[i][:], Act.Tanh, scale=t_sc[:, 0:1])
                kraw = ptiny.tile([128, 1], f32, tag=f"kraw{i}", name=f"kraw{i}")
                nc.vector.tensor_scalar(
                    out=kraw[:], in0=surp[:], scalar1=float(K_HIGH - K_LOW),
                    scalar2=float(K_LOW), op0=Alu.mult, op1=Alu.add,
                )
                km05 = ptiny.tile([128, 1], f32, tag=f"km05_{i}", name=f"km05_{i}")
                nc.vector.tensor_scalar(
                    out=km05[:], in0=kraw[:], scalar1=0.5, scalar2=None, op0=Alu.subtract
                )
                km15 = ptiny.tile([128, 1], f32, tag=f"km15_{i}", name=f"km15_{i}")
                nc.vector.tensor_scalar(
                    out=km15[:], in0=kraw[:], scalar1=1.5, scalar2=None, op0=Alu.subtract
                )
                m1 = ptiny.tile([128, 16], f32, tag=f"m1_{i}", name=f"m1_{i}")
                nc.vector.tensor_scalar(
                    out=m1[:], in0=t_iota[:], scalar1=km05[:], scalar2=None, op0=Alu.is_le
                )
                m2 = ptiny.tile([128, 16], f32, tag=f"m2_{i}", name=f"m2_{i}")
                nc.vector.tensor_scalar(
                    out=m2[:], in0=t_iota[:], scalar1=km15[:], scalar2=None, op0=Alu.is_gt
                )
                kmask = ptiny.tile([128, 16], f32, tag=f"kmask{i}", name=f"kmask{i}")
                nc.vector.tensor_tensor(out=kmask[:], in0=m1[:], in1=m2[:], op=Alu.mult)
                kmasks[i] = kmask
                kss = ptiny.tile([128, 16], f32, tag=f"kss{i}", name=f"kss{i}")
                km1 = ptiny.tile([128, 1], f32, tag=f"km1_{i}", name=f"km1_{i}")
                nc.vector.tensor_tensor(
                    out=kss[:], in0=kmask[:], in1=t_iota[:], op=Alu.mult
                )
                nc.vector.tensor_reduce(
                    out=km1[:], in_=kss[:], axis=mybir.AxisListType.X, op=Alu.add
                )
                deg = ptiny.tile([128, 1], f32, tag=f"deg{i}", name=f"deg{i}")
                nc.vector.tensor_scalar(
                    out=deg[:], in0=km1[:], scalar1=1.0, scalar2=t_trow[:, i : i + 1],
                    op0=Alu.add, op1=Alu.min,
                )
                nc.vector.tensor_scalar(
                    out=deg[:], in0=deg[:], scalar1=1.0, scalar2=None, op0=Alu.max
                )
                rdeg = ptiny.tile([128, 1], f32, tag=f"rdeg{i}", name=f"rdeg{i}")
                nc.vector.reciprocal(rdeg[:], deg[:])
                rdegs[i] = rdeg

        # ---- phase 2: sim + topk -> per-slot A (row-major bf16) ----
        # stream + lhs pools live for the whole kernel and rotate buffers, so
        # each group's loads take WAR deps on the previous group's
        # (early-freed) sim reads instead of on late epilogue readers.
        pstream = ctx.enter_context(tc.tile_pool(name="pstream", bufs=2))
        plhs = ctx.enter_context(tc.tile_pool(name="plhs", bufs=2))
        abs_ = {}
        for i in range(NSLOT):
            abs_[i] = pab.tile(
                [128, 512 * (i + 1)], bf16, tag=f"ab{i}", name=f"ab{i}"
            )
        for gidx, slots in enumerate(groups):
            gmax = max(slots)
            n_hc = gmax + 1
            with ExitStack() as sctx:
                psim = sctx.enter_context(tc.tile_pool(name=f"psim{gidx}", bufs=1))
                prep = sctx.enter_context(tc.tile_pool(name=f"prep{gidx}", bufs=1))
                pps = sctx.enter_context(
                    tc.tile_pool(name=f"pps{gidx}", bufs=3, space="PSUM")
                )
                g0r = 128 * list(slots)[0]
                lhg_hi = plhs.tile([128, KD, 512], bf16, tag="lhgh", name="lhgh")
                nc.sync.dma_start(lhg_hi[:], xrT_hi_v[:, :, g0r : g0r + 512])
                lhg_lo = plhs.tile([128, KD, 512], bf16, tag="lhgl", name="lhgl")
                nc.sync.dma_start(lhg_lo[:], xrT_lo_v[:, :, g0r : g0r + 512])
                lhs_hi, lhs_lo = {}, {}
                for k, i in enumerate(slots):
                    lhs_hi[i] = [
                        lhg_hi[:, dk, 128 * k : 128 * (k + 1)] for dk in range(KD)
                    ]
                    lhs_lo[i] = [
                        lhg_lo[:, dk, 128 * k : 128 * (k + 1)] for dk in range(KD)
                    ]
                simbufs = {}
                for k, i in enumerate(slots):
                    simbufs[i] = psim.tile(
                        [128, 512 * (i + 1)], f32, tag=f"sim{k}", name=f"sim{i}"
                    )
                for hc in range(n_hc):
                    c0 = HC * hc
                    rh = pstream.tile([128, KD, HC], bf16, tag="rsh", name="rsh")
                    nc.sync.dma_start(rh[:], xnT_hi_v[:, :, c0 : c0 + HC])
                    rl = pstream.tile([128, KD, HC], bf16, tag="rsl", name="rsl")
                    nc.sync.dma_start(rl[:], xnT_lo_v[:, :, c0 : c0 + HC])
                    for i in slots:
                        if i < hc:
                            continue
                        psg = pps.tile([128, HC], f32, tag="psg")
                        for dk in range(KD):
                            nc.tensor.matmul(
                                psg[:], lhs_hi[i][dk], rh[:, dk, :],
                                start=(dk == 0), stop=False,
                            )
                            nc.tensor.matmul(
                                psg[:], lhs_hi[i][dk], rl[:, dk, :],
                                start=False, stop=False,
                            )
                            nc.tensor.matmul(
                                psg[:], lhs_lo[i][dk], rh[:, dk, :],
                                start=False, stop=(dk == KD - 1),
                            )
                        seg = simbufs[i][:, c0 : c0 + HC]
                        if hc != i:
                            nc.scalar.activation(seg, psg[:], Act.Copy)
                            continue
                        # diagonal chunk: fuse the causal staircase mask,
                        # then run the topk/threshold chain on the DVE (the
                        # PE proceeds with the other slots' matmuls).
                        nc.vector.tensor_tensor(
                            out=seg, in0=psg[:], in1=t_dmask[:], op=Alu.add
                        )
                        W = 512 * (i + 1)
                        sb = simbufs[i]
                        top16 = ptiny.tile(
                            [128, 16], f32, tag=f"top16_{i}", name=f"top16_{i}"
                        )
                        nc.vector.max(top16[:, 0:8], sb[:])
                        rep = prep.tile(
                            [128, 512 * NSLOT], f32, tag="rep", name="rep"
                        )
                        nc.vector.match_replace(
                            rep[:, :W], top16[:, 0:8], sb[:], NEG_BIG
                        )
                        nc.vector.max(top16[:, 8:16], rep[:, :W])
                        tts = ptiny.tile([128, 16], f32, tag=f"tts{i}", name=f"tts{i}")
                        th = ptiny.tile([128, 1], f32, tag=f"th{i}", name=f"th{i}")
                        nc.vector.tensor_tensor(
                            out=tts[:], in0=top16[:], in1=kmasks[i][:], op=Alu.mult
                        )
                        nc.vector.tensor_reduce(
                            out=th[:], in_=tts[:], axis=mybir.AxisListType.X, op=Alu.add
                        )
                        nc.vector.tensor_scalar(
                            out=th[:], in0=th[:], scalar1=TH_FLOOR, scalar2=None,
                            op0=Alu.max,
                        )
                        nc.vector.tensor_scalar(
                            out=abs_[i][:], in0=sb[:], scalar1=th[:], scalar2=None,
                            op0=Alu.is_ge,
                        )

            # ---- A^T (PE transpose) + aggregation + epilogue (this group) ----
            # Two sub-passes (low slots, then high slots): the high slots'
            # topk chains still drain on the DVE while the low slots'
            # transposes + matmuls keep the PE fed.
            sidx = gidx
            with ExitStack() as mctx:
                pmsg = mctx.enter_context(tc.tile_pool(name=f"pmsg{sidx}", bufs=1))
                pxnc = mctx.enter_context(tc.tile_pool(name=f"pxnc{sidx}", bufs=3))
                pepi = mctx.enter_context(tc.tile_pool(name=f"pepi{sidx}", bufs=1))
                pat = mctx.enter_context(tc.tile_pool(name=f"pat{sidx}", bufs=1))
                ppsm = mctx.enter_context(
                    tc.tile_pool(name=f"ppsm{sidx}", bufs=1, space="PSUM")
                )
                ptr = mctx.enter_context(
                    tc.tile_pool(name=f"ptr{sidx}", bufs=2, space="PSUM")
                )
                ats, msgs, xes = {}, {}, {}
                for k, i in enumerate(slots):
                    ats[i] = pat.tile(
                        [128, 4 * (i + 1) * 128], bf16, tag=f"at{k}", name=f"at{i}"
                    )
                    msgs[i] = pmsg.tile([128, D], f32, tag=f"msg{k}", name=f"msg{i}")
                    xe = pepi.tile([128, D], f32, tag="xe", name=f"xe{i}", bufs=2)
                    nc.gpsimd.dma_start(xe[:], xrn[128 * i : 128 * (i + 1), :])
                    xes[i] = xe
                # batched A^T: PE transposes run back-to-back; scalar/DVE
                # copies drain behind them without stalling the PE queue
                for i in slots:
                    for q in range(4 * (i + 1)):
                        pt = ptr.tile([128, 128], bf16, tag="pt")
                        nc.tensor.transpose(
                            pt[:], abs_[i][:, 128 * q : 128 * (q + 1)], t_ident[:]
                        )
                        if q % 2 == 0:
                            nc.scalar.activation(
                                ats[i][:, 128 * q : 128 * (q + 1)], pt[:], Act.Copy
                            )
                        else:
                            nc.vector.tensor_copy(
                                ats[i][:, 128 * q : 128 * (q + 1)], pt[:]
                            )
                for h in range(2):
                    psms = {}
                    for k, i in enumerate(slots):
                        psms[i] = ppsm.tile(
                            [128, DH], f32, tag=f"psm{k}", name=f"psm{i}"
                        )
                    for c in range(gmax + 1):
                        xnc = pxnc.tile([128, 4, DH], bf16, tag="xnc", name="xnc")
                        nc.gpsimd.dma_start(
                            xnc[:],
                            xnat_v[:, 4 * c : 4 * (c + 1), DH * h : DH * (h + 1)],
                        )
                        for i in slots:
                            if i < c:
                                continue
                            for sub in range(4):
                                q = 4 * c + sub
                                nc.tensor.matmul(
                                    psms[i][:],
                                    ats[i][:, 128 * q : 128 * (q + 1)],
                                    xnc[:, sub, :],
                                    start=(q == 0), stop=(q == 4 * (i + 1) - 1),
                                )
                            if i != c:
                                continue
                            # slot done accumulating: evacuate + scale now
                            # msg*(1-mix)*gain/deg: TT with g2 (PSUM src), 1/deg
                            mseg = msgs[i][:, DH * h : DH * (h + 1)]
                            nc.vector.tensor_tensor(
                                out=mseg, in0=psms[i][:],
                                in1=t_g2[:, DH * h : DH * (h + 1)], op=Alu.mult,
                            )
                            nc.vector.tensor_scalar(
                                out=mseg, in0=mseg, scalar1=rdegs[i][:],
                                scalar2=None, op0=Alu.mult,
                            )
                            if h != 1:
                                continue
                            # both halves done: epilogue for this slot
                            e1 = pepi.tile([128, D], f32, tag="e1", name="e1", bufs=1)
                            nc.vector.tensor_tensor(
                                out=e1[:], in0=xes[i][:], in1=t_g1[:], op=Alu.mult
                            )
                            nc.vector.tensor_tensor(
                                out=e1[:], in0=e1[:], in1=msgs[i][:], op=Alu.add
                            )
                            nc.vector.tensor_tensor(
                                out=e1[:], in0=e1[:], in1=t_bias[:], op=Alu.add
                            )
                            g = pepi.tile([128, D], f32, tag="g", name="g", bufs=1)
                            nc.scalar.activation(g[:], e1[:], gelu_func)
                            nc.vector.tensor_scalar(
                                out=g[:], in0=g[:], scalar1=t_sc[:, 1:2],
                                scalar2=None, op0=Alu.mult,
                            )
                            nc.scalar.dma_start(out[128 * i : 128 * (i + 1), :], g[:])

    nc.compile()
    return nc


# ----------------------------------------------------------------------------
# Host-side sharding
# ----------------------------------------------------------------------------
def _softplus32(v):
    v = np.float32(v)
    return np.float32(np.log1p(np.exp(np.float64(v))))


def make_core_inputs(inputs, T=4096, D=1024):
    """Build the 8 per-core input maps from the full problem inputs."""
    import ml_dtypes

    bf = ml_dtypes.bfloat16
    x = np.ascontiguousarray(np.asarray(inputs["x"], dtype=np.float32))
    B = x.shape[0]
    NSLOT = T // 512
    R = NSLOT * 128
    f = np.float32

    mix = f(1.0 / (1.0 + np.exp(-np.float64(np.asarray(inputs["log_mix"])))))
    scale = _softplus32(np.asarray(inputs["log_scale"])) + f(0.01)
    sigma = _softplus32(np.asarray(inputs["log_sigma_raw"])) + f(0.01)
    ema_mean = np.asarray(inputs["ema_mean"], dtype=np.float32)
    ema_sq = np.asarray(inputs["ema_sq"], dtype=np.float32)
    gain = np.asarray(inputs["gain"], dtype=np.float32)
    bias = np.asarray(inputs["bias"], dtype=np.float32)

    std = np.sqrt(np.clip(ema_sq - ema_mean * ema_mean, f(1e-6), None)).astype(f)
    inv_std = (f(1.0) / std).astype(f)
    negm = (-ema_mean * inv_std).astype(f)
    g1 = (gain * mix).astype(f)
    g2 = (gain * (f(1.0) - mix)).astype(f)

    def rep(v):
        return np.ascontiguousarray(np.tile(v[None, :], (128, 1)).astype(f))

    iota16 = np.ascontiguousarray(
        np.tile(np.arange(16, dtype=f)[None, :], (128, 1))
    )
    ident = np.ascontiguousarray(np.eye(128, dtype=bf))

    # per-batch: normalized columns (hi/lo bf16) + raw bf16 rows for aggregation
    b_xnT_hi, b_xnT_lo, b_xnat = [], [], []
    for b in range(B):
        xb = x[b]
        n64 = np.linalg.norm(xb.astype(np.float64), axis=1)
        n64 = np.maximum(n64, 1e-12)
        xn = (xb.astype(np.float64) / n64[:, None]).astype(f)
        xn_hi = xn.astype(bf)
        xn_lo = (xn - xn_hi.astype(f)).astype(bf)
        b_xnT_hi.append(np.ascontiguousarray(xn_hi.T))
        b_xnT_lo.append(np.ascontiguousarray(xn_lo.T))
        b_xnat.append(np.ascontiguousarray(xb.astype(bf)))

    in_maps = []
    rows_by_core = []
    for c in range(8):
        b, j = c // 4, c % 4
        rows = np.concatenate(
            [512 * i + j + 4 * np.arange(128) for i in range(NSLOT)]
        ).astype(np.int64)
        rows_by_core.append((b, rows))
        xb = x[b]
        xr = np.ascontiguousarray(xb[rows])
        xr_hi = xr.astype(bf)
        xr_lo = (xr - xr_hi.astype(f)).astype(bf)
        m = np.arange(128)
        dmask = np.where(
            np.arange(512)[None, :] < (j + 4 * m)[:, None], f(0.0), f(NEG_BIG)
        ).astype(f)
        trow = np.stack(
            [(512 * i + j + 4 * m).astype(f) for i in range(NSLOT)], axis=1
        )
        sc = np.zeros((128, 2), f)
        sc[:, 0] = sigma / f(D)
        sc[:, 1] = scale
        in_maps.append(
            {
                "xnT_hi": b_xnT_hi[b],
                "xnT_lo": b_xnT_lo[b],
                "xrT_hi": np.ascontiguousarray(xr_hi.T),
                "xrT_lo": np.ascontiguousarray(xr_lo.T),
                "xnat": b_xnat[b],
                "xrn": xr,
                "invstd_b": rep(inv_std),
                "negm_b": rep(negm),
                "g1_b": rep(g1),
                "g2_b": rep(g2),
                "bias_b": rep(bias),
                "dmask": np.ascontiguousarray(dmask),
                "iota16": iota16,
                "trow": np.ascontiguousarray(trow),
                "sc": sc,
                "ident": ident,
            }
        )
    return in_maps, rows_by_core, (B, x.shape[1], x.shape[2])


_PROG_CACHE = {}


def kernel(**inputs):
    from concourse.bass_utils import run_bass_kernel_spmd

    x = np.asarray(inputs["x"])
    B, T, D = x.shape
    key = (T, D)
    if key not in _PROG_CACHE:
        _PROG_CACHE[key] = build_program(T=T, D=D)
    nc = _PROG_CACHE[key]
    in_maps, rows_by_core, _ = make_core_inputs(inputs, T=T, D=D)
    res = run_bass_kernel_spmd(nc, in_maps, list(range(8)))
    y = np.zeros((B, T, D), dtype=np.float32)
    for c in range(8):
        b, rows = rows_by_core[c]
        y[b, rows, :] = res.results[c]["out"]
    return y


if __name__ == "__main__":
    rng = np.random.default_rng(0)
    B, T, D = 2, 4096, 1024
    demo = {
        "x": rng.standard_normal((B, T, D)).astype(np.float32),
        "gain": np.ones(D, np.float32),
        "bias": np.zeros(D, np.float32),
        "log_mix": np.zeros((), np.float32),
        "log_scale": np.zeros((), np.float32),
        "log_sigma_raw": np.zeros((), np.float32),
        "logit_decay": np.zeros((), np.float32),
        "ema_mean": np.zeros(D, np.float32),
        "ema_sq": np.ones(D, np.float32),
    }
    y = kernel(**demo)
    print("out", y.shape, float(np.abs(y).max()))
